# revision 1
# baseline (speedup 1.0000x reference)
"""Multihead attention (B=2, S=2048, E=1024, H=16) on 8 TRN2 cores.

Sharding: tensor-parallel over heads — core c computes heads {2c, 2c+1}
(dout = 128 columns of the QKV projections) for the full sequence, then its
partial contribution to the output projection; the host sums the 8 partials
and adds the output bias.

Device layout (per core):
  activations are pre-transposed on host to x^T [E, B*S] (and rounded to
  bf16 — the bf16 matmuls would round them anyway) so the projection
  matmuls contract E on the partition dim.  QKV projections produce
  Q^T/K^T/V^T [128, 4096] in SBUF (bf16).  Attention per (batch, head)
  computes scores^T [kpos, q] tiles directly (lhsT = K^T slice,
  rhs = Q^T slice), exponentiates on the scalar engine (fp32 psum in,
  bf16 out), and multiplies by V via matmul with lhsT = [V | ones] so the
  softmax denominator falls out of the same accumulation (row 64 of the
  PSUM result).  context^T is normalized with a reciprocal + PE-replicated
  row (kept float32r), and the output projection (float32r = full fp32
  bits) contracts the 128 local head dims.

Emission order interleaves batch-0 attention with batch-1 projections and
batch-1 attention with batch-0 output projection so DMA streaming, PE,
ACT (exp) and DVE stay overlapped across the whole kernel.
"""

import numpy as np
import ml_dtypes

# Problem constants (hardcoded per the task contract).
B, S, E, H = 2, 2048, 1024, 16
D = E // H          # 64
NSEQ = B * S        # 4096
NCORES = 8
DOUT = E // NCORES  # 128 = 2 heads x 64
KE = E // 128       # 8 contraction tiles over E
SEQT = 512          # seq tile for projections / q-block for attention
NST = NSEQ // SEQT  # 8
QB = S // SEQT      # 4 q-blocks per batch
KT = S // 128       # 16 kpos tiles per batch
ISD = float(D) ** -0.5

_PROGRAM = None


# ---------------------------------------------------------------------------
# Workarounds for this walrus build: at most ONE sync wait per instruction is
# reliably accepted ("Too many sync wait commands").  (1) tile's final drain
# gets one wait per logical proc — split them over single-wait SP NOPs;
# (2) a general post-pass moves any instruction's excess waits onto
# preceding same-engine NOPs (engine program order preserves semantics).
# ---------------------------------------------------------------------------


def _install_tile_drain_patch():
    import concourse.mybir as mybir
    import concourse.tile as tile
    from concourse.tile import ScopedClock

    if getattr(tile.TileContext, "_drain_patch_installed", False):
        return

    def _patched_drain_and_barrier(self, tick_clock, wait_clock):
        nc = self.nc
        carrier = nc.sync.nop(nofuse=True)
        wait_clock.add_sem_waits(
            carrier.ins, ScopedClock({None: tick_clock.global_clock})
        )
        si = carrier.ins.sync_info
        waits = list(si.on_wait) if si and si.on_wait else []
        ups = list(si.on_update) if si and si.on_update else []
        if len(waits) > 1:
            carrier.ins.sync_info = mybir.SyncInfo(on_wait=[waits[0]], on_update=ups)
            for w in waits[1:]:
                n2 = nc.sync.nop(nofuse=True)
                n2.ins.sync_info = mybir.SyncInfo(on_wait=[w], on_update=[])
        nc.sync.drain()
        nc.all_engine_barrier()
        popped = nc._tile_sem_poison_stack.pop()
        assert popped is self._sem_poison
        nc.clear_and_free_semaphores(list(self.sems.allocated().values()))
        nc.all_engine_barrier()

    tile.TileContext._drain_and_barrier = _patched_drain_and_barrier
    tile.TileContext._drain_patch_installed = True


MAX_WAITS = 1


def _split_excess_waits(nc):
    import concourse.mybir as mybir

    for bb in nc.main_func.blocks:
        il = list(bb.instructions)
        out = []
        changed = False
        for ins in il:
            si = ins.sync_info
            waits = list(si.on_wait) if si and si.on_wait else []
            if len(waits) > MAX_WAITS:
                changed = True
                extras = waits[: len(waits) - MAX_WAITS]
                keep = waits[len(extras):]
                for i in range(0, len(extras), MAX_WAITS):
                    chunk = extras[i : i + MAX_WAITS]
                    nop = mybir.InstNoOp(
                        name=nc.get_next_instruction_name(), ins=[], outs=[]
                    )
                    nop.engine = ins.engine
                    nop.sync_info = mybir.SyncInfo(on_wait=chunk, on_update=[])
                    out.append(nop)
                ins.sync_info = mybir.SyncInfo(
                    on_wait=keep, on_update=list(si.on_update) if si.on_update else []
                )
            out.append(ins)
        if changed:
            bb.instructions = out


def _build_program():
    import concourse.bass as bass
    import concourse.mybir as mybir
    import concourse.tile as tile
    from concourse.masks import make_identity

    _install_tile_drain_patch()

    f32 = mybir.dt.float32
    f32r = mybir.dt.float32r
    bf16 = mybir.dt.bfloat16

    nc = bass.Bass("TRN2", target_bir_lowering=False, debug=False)

    # DRAM I/O (per core).  Activations/projection weights are bf16.
    xq = nc.dram_tensor("xq", [KE, 128, NSEQ], bf16, kind="ExternalInput").ap()
    xk = nc.dram_tensor("xk", [KE, 128, NSEQ], bf16, kind="ExternalInput").ap()
    xv = nc.dram_tensor("xv", [KE, 128, NSEQ], bf16, kind="ExternalInput").ap()
    wq = nc.dram_tensor("wq", [KE, 128, DOUT], bf16, kind="ExternalInput").ap()
    wk = nc.dram_tensor("wk", [KE, 128, DOUT], bf16, kind="ExternalInput").ap()
    wv = nc.dram_tensor("wv", [KE, 128, DOUT], bf16, kind="ExternalInput").ap()
    wo = nc.dram_tensor("wo", [DOUT, E], f32r, kind="ExternalInput").ap()
    bq = nc.dram_tensor("bq", [DOUT, 1], f32, kind="ExternalInput").ap()
    bk = nc.dram_tensor("bk", [DOUT, 1], f32, kind="ExternalInput").ap()
    bv = nc.dram_tensor("bv", [DOUT, 1], f32, kind="ExternalInput").ap()
    out = nc.dram_tensor("out", [NSEQ, E], f32, kind="ExternalOutput").ap()

    with tile.TileContext(nc) as tc:
        with (
            nc.allow_low_precision(reason="bf16/f32r attention pipeline"),
            tc.tile_pool(name="consts", bufs=1) as consts,
            tc.tile_pool(name="persist", bufs=1) as persist,
            tc.tile_pool(name="xstream", bufs=12) as xstream,
            tc.tile_pool(name="ptp", bufs=8) as ptp,
            tc.tile_pool(name="outp", bufs=4) as outp,
            tc.tile_pool(name="small", bufs=4) as small,
            tc.tile_pool(name="pp_ps", bufs=2, space="PSUM") as pp_ps,
            tc.tile_pool(name="sc_ps", bufs=4, space="PSUM") as sc_ps,
            tc.tile_pool(name="cx_ps", bufs=2, space="PSUM") as cx_ps,
        ):
            # ---- constants / persistent SBUF state ----
            ident_f32 = consts.tile([128, 128], f32)
            make_identity(nc, ident_f32[:])
            ident = consts.tile([128, 128], bf16)
            nc.vector.tensor_copy(ident[:], ident_f32[:])
            onesf = consts.tile([128, 1], f32)
            nc.vector.memset(onesf[:], 1.0)
            ones64 = consts.tile([1, 64], f32r)
            nc.vector.tensor_copy(ones64[:], onesf[0:1, 0:1].broadcast_to([1, 64]))

            w_sb = {}
            b_sb = {}
            for name, wdram, bdram in (("q", wq, bq), ("k", wk, bk), ("v", wv, bv)):
                wt = persist.tile([128, KE, DOUT], bf16, tag=f"w{name}")
                for k in range(KE):
                    nc.sync.dma_start(wt[:, k, :], wdram[k])
                w_sb[name] = wt
                bt = persist.tile([DOUT, 1], f32, tag=f"b{name}")
                nc.sync.dma_start(bt[:], bdram[:])
                b_sb[name] = bt
            wo_sb = persist.tile([DOUT, E], f32r, tag="wo")
            nc.sync.dma_start(wo_sb[:], wo[:])

            qt_sb = persist.tile([128, NSEQ], bf16, tag="qt")
            kt_sb = persist.tile([128, NSEQ], bf16, tag="kt")
            vt_sb = persist.tile([128, NSEQ], bf16, tag="vt")
            xT_sb = {"q": qt_sb, "k": kt_sb, "v": vt_sb}
            # [V | ones] per (kpos chunk, head): [128, 32, 2, 65] bf16
            v_sb = persist.tile([128, NSEQ // 128, 2, D + 1], bf16, tag="vn")
            nc.vector.tensor_copy(
                v_sb[:, :, :, D], onesf[:, 0:1].broadcast_to([128, NSEQ // 128, 2])
            )
            ctxT_sb = persist.tile([128, NSEQ], f32r, tag="ctxT")

            xdram = {"q": xq, "k": xk, "v": xv}

            def proj_step(st):
                sl = bass.ts(st, SEQT)
                for name in ("q", "k", "v"):
                    ps = pp_ps.tile([128, SEQT], f32, tag="pp", name=f"pp{st}{name}")
                    for k in range(KE):
                        xt = xstream.tile([128, SEQT], bf16, tag="xs", name="xt")
                        nc.sync.dma_start(xt[:], xdram[name][k, :, sl])
                        nc.tensor.matmul(
                            ps[:],
                            lhsT=w_sb[name][:, k, :],
                            rhs=xt[:],
                            start=(k == 0),
                            stop=(k == KE - 1),
                        )
                    nc.vector.tensor_scalar_add(
                        xT_sb[name][:, sl], ps[:], b_sb[name][:, 0:1]
                    )
                # transpose this slice of V^T into [V | ones] chunks
                for ci in range(st * (SEQT // 128), (st + 1) * (SEQT // 128)):
                    tp = pp_ps.tile([128, 128], bf16, tag="pp", name="tp")
                    nc.tensor.transpose(
                        tp[:], vt_sb[:, bass.ts(ci, 128)], ident[:]
                    )
                    for h in range(2):
                        nc.vector.tensor_copy(
                            v_sb[:, ci, h, 0:D], tp[:, bass.ts(h, D)]
                        )

            def attn_step(b, qb):
                qsl = bass.ds(b * S + qb * SEQT, SEQT)
                ctx = [None, None]
                for h in range(2):
                    ctx[h] = cx_ps.tile([D + 1, SEQT], f32, tag="cx", name=f"ctx{h}")
                for t in range(KT):
                    ksl = bass.ds(b * S + t * 128, 128)
                    pt = [None, None]
                    for h in range(2):
                        hsl = bass.ts(h, D)
                        sc = sc_ps.tile([128, SEQT], f32, tag="sc", name=f"sc{h}")
                        nc.tensor.matmul(
                            sc[:],
                            lhsT=kt_sb[hsl, ksl],
                            rhs=qt_sb[hsl, qsl],
                            start=True,
                            stop=True,
                        )
                        pt[h] = ptp.tile([128, SEQT], bf16, tag="pt", name=f"pt{h}")
                        nc.scalar.activation(
                            pt[h][:], sc[:], mybir.ActivationFunctionType.Exp,
                            scale=ISD,
                        )
                    for h in range(2):
                        nc.tensor.matmul(
                            ctx[h][:],
                            lhsT=v_sb[:, b * KT + t, h, :],
                            rhs=pt[h][:],
                            start=(t == 0),
                            stop=(t == KT - 1),
                        )
                for h in range(2):
                    hsl = bass.ts(h, D)
                    rec = small.tile([1, SEQT], f32r, tag="rec", name="rec")
                    nc.vector.reciprocal(rec[:], ctx[h][D : D + 1, :])
                    rrep = pp_ps.tile([D, SEQT], f32, tag="pp", name="rrep")
                    nc.tensor.matmul(
                        rrep[:], lhsT=ones64[:], rhs=rec[:], start=True, stop=True
                    )
                    ctmp = small.tile([D, SEQT], f32, tag="ctmp", name="ctmp")
                    nc.vector.tensor_copy(ctmp[:], ctx[h][0:D, :])
                    nc.vector.tensor_tensor(
                        out=ctxT_sb[hsl, qsl],
                        in0=ctmp[:],
                        in1=rrep[:],
                        op=mybir.AluOpType.mult,
                    )

            def outproj_step(m):
                ob = outp.tile([128, E], f32, tag="ob", name="ob")
                for n in range(E // SEQT):
                    ps = pp_ps.tile([128, SEQT], f32, tag="pp", name="ops")
                    nc.tensor.matmul(
                        ps[:],
                        lhsT=ctxT_sb[:, bass.ts(m, 128)],
                        rhs=wo_sb[:, bass.ts(n, SEQT)],
                        start=True,
                        stop=True,
                    )
                    nc.vector.tensor_copy(ob[:, bass.ts(n, SEQT)], ps[:])
                nc.sync.dma_start(out[bass.ts(m, 128), :], ob[:])

            # ---- emission: overlap batches ----
            for st in range(4):           # batch-0 projections
                proj_step(st)
            for qb in range(QB):          # b0 attention // b1 projections
                attn_step(0, qb)
                proj_step(4 + qb)
            for qb in range(QB):          # b1 attention // b0 out-proj
                attn_step(1, qb)
                for m in range(4 * qb, 4 * qb + 4):
                    outproj_step(m)
            for m in range(16, 32):       # b1 out-proj
                outproj_step(m)

    return nc


def _get_program():
    global _PROGRAM
    if _PROGRAM is None:
        _PROGRAM = _build_program()
    return _PROGRAM


def kernel(query, key, value, Wq, bq, Wk, bk, Wv, bv, Wo, bo):
    from concourse.bass_utils import run_bass_kernel_spmd

    nc = _get_program()
    if not getattr(nc, "_waits_split", False):
        _split_excess_waits(nc)
        nc._waits_split = True

    bf = ml_dtypes.bfloat16
    q2 = np.asarray(query, np.float32).reshape(NSEQ, E)
    k2 = np.asarray(key, np.float32).reshape(NSEQ, E)
    v2 = np.asarray(value, np.float32).reshape(NSEQ, E)
    # x^T [E, NSEQ] -> [KE, 128, NSEQ], rounded to bf16 on host (the bf16
    # matmul rounds its inputs anyway)
    xq = np.ascontiguousarray(q2.T).astype(bf).reshape(KE, 128, NSEQ)
    xk = np.ascontiguousarray(k2.T).astype(bf).reshape(KE, 128, NSEQ)
    xv = np.ascontiguousarray(v2.T).astype(bf).reshape(KE, 128, NSEQ)

    Wq = np.asarray(Wq, np.float32)
    Wk = np.asarray(Wk, np.float32)
    Wv = np.asarray(Wv, np.float32)
    Wo = np.asarray(Wo, np.float32)

    in_maps = []
    for c in range(NCORES):
        rsl = slice(DOUT * c, DOUT * (c + 1))
        in_maps.append(
            {
                "xq": xq, "xk": xk, "xv": xv,
                # lhsT for the projections: (W_c)^T [E, DOUT] -> [KE,128,DOUT]
                "wq": np.ascontiguousarray(Wq[rsl, :].T).astype(bf).reshape(KE, 128, DOUT),
                "wk": np.ascontiguousarray(Wk[rsl, :].T).astype(bf).reshape(KE, 128, DOUT),
                "wv": np.ascontiguousarray(Wv[rsl, :].T).astype(bf).reshape(KE, 128, DOUT),
                # rhs for the out-proj: rows c-range of Wo^T  [DOUT, E]
                "wo": np.ascontiguousarray(Wo[:, rsl].T),
                "bq": np.ascontiguousarray(np.asarray(bq, np.float32)[rsl]).reshape(DOUT, 1),
                "bk": np.ascontiguousarray(np.asarray(bk, np.float32)[rsl]).reshape(DOUT, 1),
                "bv": np.ascontiguousarray(np.asarray(bv, np.float32)[rsl]).reshape(DOUT, 1),
            }
        )

    res = run_bass_kernel_spmd(nc, in_maps, list(range(NCORES)), trace=False)
    acc = np.zeros((NSEQ, E), np.float32)
    for c in range(NCORES):
        acc += res.results[c]["out"]
    acc += np.asarray(bo, np.float32)[None, :]
    return acc.reshape(B, S, E)



# revision 12
# speedup vs baseline: 1.3398x; 1.3398x over previous
"""Multihead attention (B=2, S=2048, E=1024, H=16) on 8 TRN2 cores.

Sharding: tensor-parallel over heads — core c computes heads {2c, 2c+1}
(dout = 128 columns of the QKV projections) for the full sequence, then its
partial contribution to the output projection; the host sums the 8 partials
and adds the output bias.

Device layout (per core):
  activations are pre-transposed on host to x^T [E, B*S] (bf16, packed
  [128, NST, KE, SEQT] so each seq-tile streams as ONE 8KB-line DMA) so the
  projection matmuls contract E on the partition dim.  QKV projections
  produce Q^T/K^T/V^T [128, 4096] in SBUF (bf16).  Attention per
  (batch, head-pair) computes both heads' scores^T [kpos, q] into one
  2-bank PSUM tile ([128, 1024], head h in columns h*512..), exponentiates
  with a single scalar-engine op (fp32 psum in, bf16 out), and multiplies
  by V via matmul with lhsT = [V | ones] so the softmax denominator falls
  out of the same accumulation (row 64 of the PSUM result).  context^T is
  normalized with reciprocal_approx_fast + a PE-replicated row, cast to
  bf16, and the output projection runs fully in bf16 (fp32 psum), writing
  fp16 partials to HBM.

Emission order interleaves batch-0 attention with batch-1 projections and
batch-1 attention with batch-0 output projection; each attention step's
normalization is emitted after the NEXT step's first score matmuls so the
in-order PE queue never stalls on the reciprocal.
"""

import numpy as np
import ml_dtypes

# Problem constants (hardcoded per the task contract).
B, S, E, H = 2, 2048, 1024, 16
D = E // H          # 64
NSEQ = B * S        # 4096
NCORES = 8
DOUT = E // NCORES  # 128 = 2 heads x 64
KE = E // 128       # 8 contraction tiles over E
SEQT = 512          # seq tile for projections / q-block for attention
NST = NSEQ // SEQT  # 8
QB = S // SEQT      # 4 q-blocks per batch
KT = S // 128       # 16 kpos tiles per batch
ISD = float(D) ** -0.5

_PROGRAM = None


# ---------------------------------------------------------------------------
# Workarounds for this walrus build: at most ONE sync wait per instruction is
# reliably accepted ("Too many sync wait commands").  (1) tile's final drain
# gets one wait per logical proc — split them over single-wait SP NOPs;
# (2) a general post-pass moves any instruction's excess waits onto
# preceding same-engine NOPs (engine program order preserves semantics).
# ---------------------------------------------------------------------------


def _install_tile_drain_patch():
    import concourse.mybir as mybir
    import concourse.tile as tile
    from concourse.tile import ScopedClock

    if getattr(tile.TileContext, "_drain_patch_installed", False):
        return

    def _patched_drain_and_barrier(self, tick_clock, wait_clock):
        nc = self.nc
        carrier = nc.sync.nop(nofuse=True)
        wait_clock.add_sem_waits(
            carrier.ins, ScopedClock({None: tick_clock.global_clock})
        )
        si = carrier.ins.sync_info
        waits = list(si.on_wait) if si and si.on_wait else []
        ups = list(si.on_update) if si and si.on_update else []
        if len(waits) > 1:
            carrier.ins.sync_info = mybir.SyncInfo(on_wait=[waits[0]], on_update=ups)
            for w in waits[1:]:
                n2 = nc.sync.nop(nofuse=True)
                n2.ins.sync_info = mybir.SyncInfo(on_wait=[w], on_update=[])
        nc.sync.drain()
        nc.all_engine_barrier()
        popped = nc._tile_sem_poison_stack.pop()
        assert popped is self._sem_poison
        nc.clear_and_free_semaphores(list(self.sems.allocated().values()))
        nc.all_engine_barrier()

    tile.TileContext._drain_and_barrier = _patched_drain_and_barrier
    tile.TileContext._drain_patch_installed = True


MAX_WAITS = 1


def _split_excess_waits(nc):
    import concourse.mybir as mybir

    for bb in nc.main_func.blocks:
        il = list(bb.instructions)
        out = []
        changed = False
        for ins in il:
            si = ins.sync_info
            waits = list(si.on_wait) if si and si.on_wait else []
            if len(waits) > MAX_WAITS:
                changed = True
                extras = waits[: len(waits) - MAX_WAITS]
                keep = waits[len(extras):]
                for i in range(0, len(extras), MAX_WAITS):
                    chunk = extras[i : i + MAX_WAITS]
                    nop = mybir.InstNoOp(
                        name=nc.get_next_instruction_name(), ins=[], outs=[]
                    )
                    nop.engine = ins.engine
                    nop.sync_info = mybir.SyncInfo(on_wait=chunk, on_update=[])
                    out.append(nop)
                ins.sync_info = mybir.SyncInfo(
                    on_wait=keep, on_update=list(si.on_update) if si.on_update else []
                )
            out.append(ins)
        if changed:
            bb.instructions = out


def _build_program():
    import concourse.bass as bass
    import concourse.mybir as mybir
    import concourse.tile as tile
    from concourse.masks import make_identity

    _install_tile_drain_patch()

    f32 = mybir.dt.float32
    f32r = mybir.dt.float32r
    bf16 = mybir.dt.bfloat16
    fp16 = mybir.dt.float16

    nc = bass.Bass("TRN2", target_bir_lowering=False, debug=False)

    # DRAM I/O (per core).  x packed [128, NST, KE, SEQT] so one seq-tile is
    # a single DMA with 8KB contiguous per partition; weights packed
    # [128, KE, DOUT] (2KB lines).
    xq = nc.dram_tensor("xq", [128, NST, KE, SEQT], bf16, kind="ExternalInput").ap()
    xk = nc.dram_tensor("xk", [128, NST, KE, SEQT], bf16, kind="ExternalInput").ap()
    xv = nc.dram_tensor("xv", [128, NST, KE, SEQT], bf16, kind="ExternalInput").ap()
    wq = nc.dram_tensor("wq", [128, KE, DOUT], bf16, kind="ExternalInput").ap()
    wk = nc.dram_tensor("wk", [128, KE, DOUT], bf16, kind="ExternalInput").ap()
    wv = nc.dram_tensor("wv", [128, KE, DOUT], bf16, kind="ExternalInput").ap()
    wo = nc.dram_tensor("wo", [DOUT, E], bf16, kind="ExternalInput").ap()
    bq = nc.dram_tensor("bq", [DOUT, 1], f32, kind="ExternalInput").ap()
    bk = nc.dram_tensor("bk", [DOUT, 1], f32, kind="ExternalInput").ap()
    bv = nc.dram_tensor("bv", [DOUT, 1], f32, kind="ExternalInput").ap()
    out = nc.dram_tensor("out", [NSEQ, E], fp16, kind="ExternalOutput").ap()

    with tile.TileContext(nc) as tc:
        with (
            nc.allow_low_precision(reason="bf16 attention pipeline"),
            tc.tile_pool(name="consts", bufs=1) as consts,
            tc.tile_pool(name="persist", bufs=1) as persist,
            tc.tile_pool(name="xstream", bufs=6) as xstream,
            tc.tile_pool(name="ptp", bufs=4) as ptp,
            tc.tile_pool(name="outp", bufs=4) as outp,
            tc.tile_pool(name="small", bufs=8) as small,
            tc.tile_pool(name="pp_ps", bufs=2, space="PSUM") as pp_ps,
            tc.tile_pool(name="sc_ps", bufs=2, space="PSUM") as sc_ps,
            tc.tile_pool(name="cx_ps", bufs=2, space="PSUM") as cx_ps,
        ):
            # ---- weights / first x tiles, in the order compute needs them ----
            w_sb = {}
            b_sb = {}
            xdram = {"q": xq, "k": xk, "v": xv}
            xtiles = {}

            def load_w(name, wdram, bdram):
                wt = persist.tile([128, KE, DOUT], bf16, tag=f"w{name}", name=f"w{name}")
                nc.sync.dma_start(wt[:], wdram[:])
                w_sb[name] = wt
                bt = persist.tile([DOUT, 1], f32, tag=f"b{name}", name=f"b{name}")
                nc.sync.dma_start(bt[:], bdram[:])
                b_sb[name] = bt

            def load_x(name, st):
                xt = xstream.tile([128, KE, SEQT], bf16, tag="xs", name="xt")
                nc.sync.dma_start(xt[:], xdram[name][:, st, :, :])
                xtiles[(name, st)] = xt

            load_w("q", wq, bq)
            load_x("q", 0)
            load_w("k", wk, bk)
            load_x("k", 0)
            load_w("v", wv, bv)
            load_x("v", 0)

            # ---- constants / persistent SBUF state ----
            ident_f32 = consts.tile([128, 128], f32)
            make_identity(nc, ident_f32[:])
            ident = consts.tile([128, 128], bf16)
            nc.vector.tensor_copy(ident[:], ident_f32[:])
            onesf = consts.tile([128, 1], f32)
            nc.vector.memset(onesf[:], 1.0)
            # denominator-replication expander: out[p] = rec[64 * (p // 64)]
            # (partition bases must be 0/32/64, so the two source rows sit at
            # partitions 0 and 64)
            expand_f = consts.tile([D + 1, 128], f32)
            nc.vector.memset(expand_f[:], 0.0)
            nc.vector.memset(expand_f[0:1, 0:D], 1.0)
            nc.vector.memset(expand_f[D : D + 1, D:128], 1.0)
            expand = consts.tile([D + 1, 128], f32r)
            nc.vector.tensor_copy(expand[:], expand_f[:])

            wo_sb = persist.tile([DOUT, E], bf16, tag="wo")
            nc.sync.dma_start(wo_sb[:], wo[:])

            qt_sb = persist.tile([128, NSEQ], bf16, tag="qt")
            kt_sb = persist.tile([128, NSEQ], bf16, tag="kt")
            vt_sb = persist.tile([128, NSEQ], bf16, tag="vt")
            xT_sb = {"q": qt_sb, "k": kt_sb, "v": vt_sb}
            # [V | ones] per (kpos chunk, head): [128, 32, 2, 65] bf16
            v_sb = persist.tile([128, NSEQ // 128, 2, D + 1], bf16, tag="vn")
            nc.vector.tensor_copy(
                v_sb[:, :, :, D], onesf[:, 0:1].broadcast_to([128, NSEQ // 128, 2])
            )
            ctxT_sb = persist.tile([128, NSEQ], bf16, tag="ctxT")

            def proj_step(st):
                sl = bass.ts(st, SEQT)
                if (("q", st)) not in xtiles:
                    load_x("q", st)
                    load_x("k", st)
                    load_x("v", st)
                # prefetch next seq-tile's x while this one computes
                if st + 1 < NST and ("q", st + 1) not in xtiles:
                    load_x("q", st + 1)
                    load_x("k", st + 1)
                    load_x("v", st + 1)
                for name in ("q", "k", "v"):
                    xt = xtiles.pop((name, st))
                    ps = pp_ps.tile([128, SEQT], f32, tag="pp", name=f"pp{st}{name}")
                    for k in range(KE):
                        nc.tensor.matmul(
                            ps[:],
                            lhsT=w_sb[name][:, k, :],
                            rhs=xt[:, k, :],
                            start=(k == 0),
                            stop=(k == KE - 1),
                        )
                    nc.vector.tensor_scalar_add(
                        xT_sb[name][:, sl], ps[:], b_sb[name][:, 0:1]
                    )
                # transpose this slice of V^T into [V | ones] chunks
                for ci in range(st * (SEQT // 128), (st + 1) * (SEQT // 128)):
                    tp = pp_ps.tile([128, 128], bf16, tag="pp", name="tp")
                    nc.tensor.transpose(
                        tp[:], vt_sb[:, bass.ts(ci, 128)], ident[:]
                    )
                    for h in range(2):
                        nc.vector.tensor_copy(
                            v_sb[:, ci, h, 0:D], tp[:, bass.ts(h, D)]
                        )

            def attn_step(b, qb, fin_fast_prev, fin_slow_prev):
                qsl = bass.ds(b * S + qb * SEQT, SEQT)
                ctx = [None, None]
                for h in range(2):
                    ctx[h] = cx_ps.tile([D + 1, SEQT], f32, tag="cx", name=f"ctx{h}")
                for t in range(KT):
                    ksl = bass.ds(b * S + t * 128, 128)
                    # both heads' scores into one 2-bank psum tile
                    sc = sc_ps.tile([128, 2 * SEQT], f32, tag="sc", name="sc")
                    for h in range(2):
                        hsl = bass.ts(h, D)
                        nc.tensor.matmul(
                            sc[:, bass.ts(h, SEQT)],
                            lhsT=kt_sb[hsl, ksl],
                            rhs=qt_sb[hsl, qsl],
                            start=True,
                            stop=True,
                        )
                    if t == 0 and fin_fast_prev is not None:
                        fin_fast_prev()
                    if t == 8 and fin_slow_prev is not None:
                        fin_slow_prev()
                    pt = ptp.tile([128, 2 * SEQT], bf16, tag="pt", name="pt")
                    nc.scalar.activation(
                        pt[:], sc[:], mybir.ActivationFunctionType.Exp, scale=ISD
                    )
                    for h in range(2):
                        nc.tensor.matmul(
                            ctx[h][:],
                            lhsT=v_sb[:, b * KT + t, h, :],
                            rhs=pt[:, bass.ts(h, SEQT)],
                            start=(t == 0),
                            stop=(t == KT - 1),
                        )

                ctmp = [None, None]

                def fin_fast():
                    # one copy per head frees the ctx psum banks quickly so
                    # the next step's PV matmuls aren't blocked
                    for h in range(2):
                        ctmp[h] = small.tile([D + 1, SEQT], f32, tag="ctmp", name="ctmp")
                        nc.vector.tensor_copy(ctmp[h][:], ctx[h][:])

                def fin_slow():
                    # batched reciprocal of both heads' denominators, parked
                    # at partitions 0 and 64 (legal AP bases); rows 1..63 are
                    # memset to 1.0 so the full-tile reciprocal reads no
                    # garbage and the expander contracts them against zeros
                    den = small.tile([D + 1, SEQT], f32, tag="den", name="den")
                    nc.vector.memset(den[:], 1.0)
                    nc.vector.tensor_copy(den[0:1, :], ctmp[0][D : D + 1, :])
                    nc.vector.tensor_copy(den[D : D + 1, :], ctmp[1][D : D + 1, :])
                    rec = small.tile([D + 1, SEQT], f32r, tag="rec", name="rec")
                    nc.vector.reciprocal(rec[:], den[:])
                    # replicate both heads' 1/denom rows in one matmul
                    rrep = pp_ps.tile([128, SEQT], f32, tag="pp", name="rrep")
                    nc.tensor.matmul(
                        rrep[:], lhsT=expand[:], rhs=rec[:], start=True, stop=True
                    )
                    for h in range(2):
                        hsl = bass.ts(h, D)
                        nc.vector.tensor_tensor(
                            out=ctxT_sb[hsl, qsl],
                            in0=ctmp[h][0:D, :],
                            in1=rrep[hsl, :],
                            op=mybir.AluOpType.mult,
                        )

                return fin_fast, fin_slow

            def outproj_step(m):
                ob = outp.tile([128, E], fp16, tag="ob", name="ob")
                for n in range(E // SEQT):
                    ps = pp_ps.tile([128, SEQT], f32, tag="pp", name="ops")
                    nc.tensor.matmul(
                        ps[:],
                        lhsT=ctxT_sb[:, bass.ts(m, 128)],
                        rhs=wo_sb[:, bass.ts(n, SEQT)],
                        start=True,
                        stop=True,
                    )
                    nc.vector.tensor_copy(ob[:, bass.ts(n, SEQT)], ps[:])
                nc.sync.dma_start(out[bass.ts(m, 128), :], ob[:])

            # ---- emission: overlap batches; normalization of attention
            # step i is emitted inside step i+1 (psum-freeing copies after
            # its first score matmuls, the reciprocal chain at t=8) so the
            # PE never idles waiting on the reciprocal ----
            for st in range(4):           # batch-0 projections
                proj_step(st)
            ff = fs = None
            for qb in range(QB):          # b0 attention // b1 projections
                ff, fs = attn_step(0, qb, ff, fs)
                proj_step(4 + qb)
            # b1 attention // out-proj of ready blocks
            ready = {0: [0, 1, 2, 3], 1: [4, 5, 6, 7, 16, 17],
                     2: [8, 9, 10, 11, 18, 19], 3: [12, 13]}
            for qb in range(QB):
                ff, fs = attn_step(1, qb, ff, fs)
                for m in ready[qb]:
                    outproj_step(m)
            # last normalization (PE work from ready out-proj blocks covers
            # the reciprocal latency), then the remaining blocks
            ff()
            for m in [14, 15, 20, 21]:
                outproj_step(m)
            fs()
            for m in [22, 23] + list(range(24, 32)):
                outproj_step(m)

    return nc


def _get_program():
    global _PROGRAM
    if _PROGRAM is None:
        _PROGRAM = _build_program()
    return _PROGRAM


def kernel(query, key, value, Wq, bq, Wk, bk, Wv, bv, Wo, bo):
    from concourse.bass_utils import run_bass_kernel_spmd

    nc = _get_program()
    if not getattr(nc, "_waits_split", False):
        _split_excess_waits(nc)
        nc._waits_split = True

    bf = ml_dtypes.bfloat16
    q2 = np.asarray(query, np.float32).reshape(NSEQ, E)
    k2 = np.asarray(key, np.float32).reshape(NSEQ, E)
    v2 = np.asarray(value, np.float32).reshape(NSEQ, E)

    # x^T [E, NSEQ] -> [128, NST, KE, SEQT] (partition-major, seq-tile
    # contiguous), rounded to bf16 on host (the bf16 matmul rounds anyway)
    def pack_x(x2):
        xT = np.ascontiguousarray(x2.T).astype(bf)
        return np.ascontiguousarray(
            xT.reshape(KE, 128, NST, SEQT).transpose(1, 2, 0, 3)
        )

    xqh = pack_x(q2)
    xkh = pack_x(k2)
    xvh = pack_x(v2)

    Wq = np.asarray(Wq, np.float32)
    Wk = np.asarray(Wk, np.float32)
    Wv = np.asarray(Wv, np.float32)
    Wo = np.asarray(Wo, np.float32)

    def pack_w(W, rsl):
        # lhsT for the projections: (W_c)^T [E, DOUT] -> [128, KE, DOUT]
        wT = np.ascontiguousarray(W[rsl, :].T).astype(bf)
        return np.ascontiguousarray(wT.reshape(KE, 128, DOUT).transpose(1, 0, 2))

    in_maps = []
    for c in range(NCORES):
        rsl = slice(DOUT * c, DOUT * (c + 1))
        in_maps.append(
            {
                "xq": xqh, "xk": xkh, "xv": xvh,
                "wq": pack_w(Wq, rsl),
                "wk": pack_w(Wk, rsl),
                "wv": pack_w(Wv, rsl),
                # rhs for the out-proj: rows c-range of Wo^T  [DOUT, E]
                "wo": np.ascontiguousarray(Wo[:, rsl].T).astype(bf),
                "bq": np.ascontiguousarray(np.asarray(bq, np.float32)[rsl]).reshape(DOUT, 1),
                "bk": np.ascontiguousarray(np.asarray(bk, np.float32)[rsl]).reshape(DOUT, 1),
                "bv": np.ascontiguousarray(np.asarray(bv, np.float32)[rsl]).reshape(DOUT, 1),
            }
        )

    res = run_bass_kernel_spmd(nc, in_maps, list(range(NCORES)), trace=False)
    acc = np.zeros((NSEQ, E), np.float32)
    for c in range(NCORES):
        acc += res.results[c]["out"].astype(np.float32)
    acc += np.asarray(bo, np.float32)[None, :]
    return acc.reshape(B, S, E)
